# revision 14
# baseline (speedup 1.0000x reference)
"""Trainium2 Bass kernel for LinearAttention+APE (depthwise-separable conv QKV,
4x4-maxpooled K/V, softmax attention with additive positional bias, dsconv out).

Sharding: data-parallel over batch B=8 across 8 NeuronCores (1 image each).

Per-core pipeline (matmuls fp16 on PE, fp32 PSUM accumulation):
  x -> depthwise 3x3 (DVE/GPSIMD fused mul-add taps on padded buffer)
    -> pointwise 256->1536 (PE)  [channel order permuted to (head, dhead)]
    -> q kept full-res; k,v max-pooled 4x4 (DVE windowed reduce_max)
  per head: S = qT k (PE, i-on-partitions), exp on ACT (softmax scale folded
    into q weights; no max-subtraction needed: |S|<~2), multiply by
    host-precomputed exp(ape*scale) with row-sums in one DVE
    tensor_tensor_reduce, normalize (DVE tensor_scalar), DMA attn out,
    DMA-transpose readback, attn@v (PE) -> out dsconv (taps + PE 1x1).
"""

import sys

import numpy as np

sys.path.insert(0, "/opt/trn_rl_repo")

B, DIM, HEADS, DHEAD, RED = 8, 256, 8, 64, 16
H = W = 64
INNER = HEADS * DHEAD  # 512
N = H * W  # 4096
R2 = RED * RED  # 256
SCALE = DHEAD ** (-0.5)

PADH, PADW = H + 2, W + 3  # 66 rows x 67 col-stride

# new channel n = head*64 + d  <- old channel d*8 + head
_PERM = (np.arange(INNER) % DHEAD) * HEADS + (np.arange(INNER) // DHEAD)

_CACHE = {}


def _build_nc():
    import concourse.bass as bass
    import concourse.tile as tile
    from concourse import bacc
    from concourse import mybir
    from concourse.masks import make_identity

    f16 = mybir.dt.float16
    f32 = mybir.dt.float32
    X = mybir.AxisListType.X
    mult = mybir.AluOpType.mult
    add = mybir.AluOpType.add
    amax = mybir.AluOpType.max
    Exp = mybir.ActivationFunctionType.Exp

    nc = bacc.Bacc("TRN2")

    x_d = nc.declare_dram_parameter("x", [DIM, N], f16, isOutput=False)
    wqdw_d = nc.declare_dram_parameter("wqdw", [DIM, 9], f32, isOutput=False)
    wqpwT_d = nc.declare_dram_parameter("wqpwT", [DIM, 3 * INNER], f16, isOutput=False)
    wodw_d = nc.declare_dram_parameter("wodw", [INNER, 9], f32, isOutput=False)
    wopwT_d = nc.declare_dram_parameter("wopwT", [INNER, DIM], f16, isOutput=False)
    apesc_d = nc.declare_dram_parameter("apesc", [N, R2], f16, isOutput=False)
    attn_d = nc.declare_dram_parameter("attn", [HEADS, N, R2], f16, isOutput=True)
    out_d = nc.declare_dram_parameter("out", [DIM, N], f16, isOutput=True)

    def depthwise(eng, xpad, wcol, out_view):
        """9-tap 3x3 depthwise conv: out[c,i,j] = sum_t w[c,t]*xpad[c,i+di,j+dj]"""
        for t in range(9):
            di, dj = t // 3, t % 3
            shift = xpad[:, di : di + H, dj : dj + W]
            if t == 0:
                eng.tensor_scalar(
                    out=out_view, in0=shift, scalar1=wcol(t), scalar2=None, op0=mult
                )
            else:
                eng.scalar_tensor_tensor(
                    out=out_view, in0=shift, scalar=wcol(t), in1=out_view,
                    op0=mult, op1=add,
                )

    with tile.TileContext(nc) as tc:
        with tc.tile_pool(name="persist", bufs=1) as persist, tc.tile_pool(
            name="small", bufs=8
        ) as smallp, tc.tile_pool(name="psum", bufs=6, space="PSUM") as psum:
            # persistent across phases
            wodw_st = persist.tile([128, 4, 9], f32, tag="wodw_st")
            nc.sync.dma_start(wodw_st, wodw_d.rearrange("(t p) k -> p t k", p=128))
            wodw_sb = persist.tile([128, 4, 9], f32, tag="wodw")
            nc.vector.tensor_copy(wodw_sb, wodw_st)
            wopw_sb = persist.tile([128, 4, DIM], f16, tag="wopw")
            nc.sync.dma_start(wopw_sb, wopwT_d.rearrange("(t p) m -> p t m", p=128))
            apesc_sb = persist.tile([128, 32, R2], f16, tag="apesc")
            nc.sync.dma_start(
                apesc_sb, apesc_d.rearrange("(c p) j -> p c j", p=128)
            )
            ident = persist.tile([128, 128], f16, tag="ident")
            make_identity(nc, ident)

            q_sb = [persist.tile([128, N], f16, tag=f"q{i}", name=f"q{i}") for i in range(4)]
            kp = [persist.tile([128, R2], f16, tag=f"kp{i}", name=f"kp{i}") for i in range(8)]
            vT = [persist.tile([128, 2, 128], f16, tag=f"vT{i}", name=f"vT{i}") for i in range(4)]

            # ================= phase 1: qkv dsconv =================
            with tc.tile_pool(name="ph1", bufs=2) as ph1, tc.tile_pool(
                name="kvh", bufs=1
            ) as kvhp:
                wqdw_st = ph1.tile([128, 2, 9], f32, tag="wqdw_st")
                nc.sync.dma_start(
                    wqdw_st, wqdw_d.rearrange("(t p) k -> p t k", p=128)
                )
                wqdw_sb = ph1.tile([128, 2, 9], f32, tag="wqdw")
                nc.vector.tensor_copy(wqdw_sb, wqdw_st)
                wqpw_sb = ph1.tile([128, 2, 3 * INNER], f16, tag="wqpw")
                nc.sync.dma_start(
                    wqpw_sb, wqpwT_d.rearrange("(t p) m -> p t m", p=128)
                )

                y_dw = []
                for ct in range(2):
                    xpad = ph1.tile([128, PADH, PADW], f16, tag="xpad")
                    nc.vector.memset(xpad, 0.0)
                    nc.sync.dma_start(
                        xpad[:, 1 : 1 + H, 1 : 1 + W],
                        x_d.rearrange("(t p) (a b) -> p t a b", p=128, a=H)[:, ct],
                    )
                    y = ph1.tile([128, H, W], f16, tag="ydw")
                    depthwise(
                        nc.vector, xpad, lambda t, ct=ct: wqdw_sb[:, ct, t : t + 1], y
                    )
                    y_dw.append(y.rearrange("p a b -> p (a b)"))

                kh = [kvhp.tile([128, 1024], f16, tag=f"kvh{i}", name=f"kvh{i}") for i in range(8)]
                for mt in range(12):
                    for st in range(8):
                        ps = psum.tile([128, 512], f32, tag="ps")
                        for kt in range(2):
                            nc.tensor.matmul(
                                ps,
                                lhsT=wqpw_sb[:, kt, mt * 128 : (mt + 1) * 128],
                                rhs=y_dw[kt][:, st * 512 : (st + 1) * 512],
                                start=(kt == 0),
                                stop=(kt == 1),
                            )
                        if mt < 4:  # q
                            nc.scalar.copy(q_sb[mt][:, st * 512 : (st + 1) * 512], ps)
                        else:  # k/v horizontal 4->1 max pool
                            nc.vector.tensor_reduce(
                                out=kh[mt - 4][:, st * 128 : (st + 1) * 128],
                                in_=ps.rearrange("p (a b) -> p a b", b=4),
                                axis=X,
                                op=amax,
                            )
                # vertical 4->1 max pool
                for i in range(8):
                    nc.vector.tensor_reduce(
                        out=kp[i].rearrange("p (ig jb) -> p ig jb", jb=16),
                        in_=kh[i].rearrange(
                            "p (ig ii jb) -> p ig jb ii", ig=16, ii=4, jb=16
                        ),
                        axis=X,
                        op=amax,
                    )
                # v^T via PE transpose
                for vt in range(4):
                    for jt in range(2):
                        tp = psum.tile([128, 128], f16, tag="ps")
                        nc.tensor.transpose(
                            tp, kp[4 + vt][:, jt * 128 : (jt + 1) * 128], ident
                        )
                        nc.scalar.copy(vT[vt][:, jt, :], tp)

            # ================= phase 2: attention per head-pair =================
            with tc.tile_pool(name="zpadp", bufs=4) as zpadp:
                zpad = []
                for hp in range(4):
                    zp = zpadp.tile([128, PADH, PADW], f16, tag="zpad")
                    nc.vector.memset(zp, 0.0)
                    zpad.append(zp)

                with tc.tile_pool(name="heads", bufs=2) as headp:
                    for hp in range(4):
                        aT_pair = []
                        for par in range(2):
                            h = 2 * hp + par
                            exps = headp.tile([128, 32, R2], f16, tag="exps")
                            sums = smallp.tile([128, 32], f32, tag="sums")
                            for ic in range(32):
                                sps = psum.tile([128, R2], f32, tag="ps")
                                nc.tensor.matmul(
                                    sps,
                                    lhsT=q_sb[hp][
                                        par * 64 : (par + 1) * 64,
                                        ic * 128 : (ic + 1) * 128,
                                    ],
                                    rhs=kp[hp][par * 64 : (par + 1) * 64, :],
                                    start=True,
                                    stop=False,
                                )
                                nc.tensor.matmul(
                                    sps,
                                    lhsT=ident,
                                    rhs=apesc_sb[:, ic, :],
                                    start=False,
                                    stop=True,
                                )
                                nc.scalar.activation(
                                    exps[:, ic, :], sps, func=Exp,
                                    accum_out=sums[:, ic : ic + 1],
                                )
                            recip = smallp.tile([128, 32], f32, tag="recip")
                            nc.vector.reciprocal(recip, sums)
                            for ic in range(32):
                                nc.vector.tensor_scalar(
                                    out=exps[:, ic, :],
                                    in0=exps[:, ic, :],
                                    scalar1=recip[:, ic : ic + 1],
                                    scalar2=None,
                                    op0=mult,
                                )
                            nc.sync.dma_start(
                                attn_d[h].rearrange("(c p) j -> p c j", p=128), exps
                            )
                            aT = headp.tile([128, 2, N], f16, tag="aT")
                            for jt in range(2):
                                nc.sync.dma_start(
                                    out=aT[:, jt, :],
                                    in_=attn_d[h][:, jt * 128 : (jt + 1) * 128],
                                    transpose=True,
                                )
                            aT_pair.append(aT)

                        # attn @ v -> z (channels-on-partitions) into padded buf
                        for st in range(8):
                            zps = psum.tile([128, 512], f32, tag="ps")
                            for par in range(2):
                                for jt in range(2):
                                    nc.tensor.matmul(
                                        zps[par * 64 : (par + 1) * 64, :],
                                        lhsT=vT[hp][:, jt, par * 64 : (par + 1) * 64],
                                        rhs=aT_pair[par][
                                            :, jt, st * 512 : (st + 1) * 512
                                        ],
                                        start=(jt == 0),
                                        stop=(jt == 1),
                                    )
                            nc.scalar.copy(
                                zpad[hp][:, 1 + st * 8 : 1 + (st + 1) * 8, 1 : 1 + W],
                                zps.rearrange("p (r c) -> p r c", c=W),
                            )

                # ================= phase 3: out dsconv =================
                with tc.tile_pool(name="ph3", bufs=4) as ph3:
                    z_dw = []
                    for ct in range(4):
                        y = ph3.tile([128, H, W], f16, tag="ydw")
                        depthwise(
                            nc.vector, zpad[ct], lambda t, ct=ct: wodw_sb[:, ct, t : t + 1], y
                        )
                        z_dw.append(y.rearrange("p a b -> p (a b)"))

                    for mt in range(2):
                        for st in range(8):
                            ps = psum.tile([128, 512], f32, tag="ps")
                            for kt in range(4):
                                nc.tensor.matmul(
                                    ps,
                                    lhsT=wopw_sb[:, kt, mt * 128 : (mt + 1) * 128],
                                    rhs=z_dw[kt][:, st * 512 : (st + 1) * 512],
                                    start=(kt == 0),
                                    stop=(kt == 3),
                                )
                            ot = ph3.tile([128, 512], f16, tag="ot")
                            nc.scalar.copy(ot, ps)
                            nc.sync.dma_start(
                                out_d.rearrange("(t p) n -> p t n", p=128)[
                                    :, mt, st * 512 : (st + 1) * 512
                                ],
                                ot,
                            )

    nc.finalize()
    return nc


def _prep_inputs(x, w_qkv_dw, w_qkv_pw, w_out_dw, w_out_pw, ape):
    """Host-side preprocessing -> per-core input maps (numpy)."""
    f16 = np.float16
    wq_pw = np.asarray(w_qkv_pw, dtype=np.float32).reshape(3 * INNER, DIM)
    wq = wq_pw[0:INNER][_PERM] * SCALE  # fold softmax scale into q
    wk = wq_pw[INNER : 2 * INNER][_PERM]
    wv = wq_pw[2 * INNER :][_PERM]
    wqpwT = np.concatenate([wq, wk, wv], axis=0).T.copy()  # [DIM, 1536]

    wqdw = np.asarray(w_qkv_dw, dtype=np.float32).reshape(DIM, 9)
    wodw = np.asarray(w_out_dw, dtype=np.float32).reshape(INNER, 9)[_PERM]
    wopwT = (
        np.asarray(w_out_pw, dtype=np.float32).reshape(DIM, INNER)[:, _PERM].T.copy()
    )  # [INNER, DIM]

    apesc = np.asarray(ape, dtype=np.float32).reshape(N, R2) * SCALE

    shared = {
        "wqdw": wqdw.astype(np.float32),
        "wqpwT": wqpwT.astype(f16),
        "wodw": wodw.astype(np.float32),
        "wopwT": wopwT.astype(f16),
        "apesc": apesc.astype(f16),
    }
    xs = np.asarray(x, dtype=np.float32).reshape(B, DIM, N).astype(f16)
    return [dict(shared, x=xs[b]) for b in range(B)]


def kernel(x, w_qkv_dw, w_qkv_pw, w_out_dw, w_out_pw, ape):
    from concourse.bass_utils import run_bass_kernel_spmd

    if "nc" not in _CACHE:
        _CACHE["nc"] = _build_nc()
    nc = _CACHE["nc"]

    in_maps = _prep_inputs(x, w_qkv_dw, w_qkv_pw, w_out_dw, w_out_pw, ape)
    res = run_bass_kernel_spmd(nc, in_maps, list(range(B)))

    outs = np.stack([r["out"].astype(np.float32) for r in res.results])
    attns = np.stack([r["attn"].astype(np.float32) for r in res.results])
    out_full = outs.reshape(B, DIM, H, W)
    attn_full = attns.reshape(B, HEADS, N, R2)
    return out_full, attn_full
